# revision 4
# baseline (speedup 1.0000x reference)
"""Trainium2 Bass kernel for the scene-graph relation predictor.

Math (reference):
    er   = (edge_ctx @ W_pe + b_pe)            # [N_obj, 1024]
    head = er[:, :512]; tail = er[:, 512:]
    prod = [head[i], tail[j]]                  # [N_rel, 1024]
    gate = prod @ W_pc + b_pc
    out  = (gate * union) @ W_ec + b_ec + prod @ W_cl + b_cl

Device algebra: fold W_pe into W_pc / W_cl on the host so the kernel only
needs raw edge_ctx rows:
    gate = edge[i] @ W1 + edge[j] @ W2 + bg    with W12 = [[W1],[W2]]
    out  = (gate * union) @ W_ec + edge[i] @ Wc1 + edge[j] @ Wc2 + bl

Sharding: data-parallel over the relation dim across 8 cores. edge_ctx and
the (combined) weights are replicated; union_feat / pair_idx / output rows
are split. All per-relation tensors live K-major ([feature, rel]) on chip:
gathers use dma_gather(transpose=True), union is pre-transposed host-side,
so no on-chip transposes are needed anywhere. Everything streams through
the tensor engine in bf16 with fp32 PSUM accumulation.
"""
import numpy as np
import ml_dtypes

import concourse.bass as bass
import concourse.mybir as mybir
import concourse.tile as tile
from concourse import bacc
from concourse.bass import ts, ds
from concourse.bass_utils import run_bass_kernel_spmd

bf16 = ml_dtypes.bfloat16

N_OBJ = 20000
HID = 512
REP = 1024
NCLS = 51
NCLS_PAD = 64
N_REL = 150000
NCORES = 8
SHARD = N_REL // NCORES          # 18750
R = 512                          # relations per tile
T = (SHARD + R - 1) // R         # 37 tiles
PAD = T * R                      # 18944 padded relations per core
KC = REP // 128                  # 8 contraction chunks of the 1024 dim
GF = REP // 128                  # 8 gate-feature chunks


def _build():
    nc = bacc.Bacc(None, target_bir_lowering=False)
    f32 = mybir.dt.float32
    b16 = mybir.dt.bfloat16

    edge = nc.dram_tensor("edge", [N_OBJ, HID], b16, kind="ExternalInput")
    hidx = nc.dram_tensor("hidx", [128, PAD // 16], mybir.dt.int16, kind="ExternalInput")
    tidx = nc.dram_tensor("tidx", [128, PAD // 16], mybir.dt.int16, kind="ExternalInput")
    union_t = nc.dram_tensor("union_t", [T, 128, GF, R], b16, kind="ExternalInput")
    w12 = nc.dram_tensor("w12", [REP, REP], b16, kind="ExternalInput")
    wec = nc.dram_tensor("wec", [REP, NCLS_PAD], b16, kind="ExternalInput")
    wc12 = nc.dram_tensor("wc12", [REP, NCLS_PAD], b16, kind="ExternalInput")
    bg = nc.dram_tensor("bg", [128, GF], f32, kind="ExternalInput")
    bl = nc.dram_tensor("bl", [NCLS_PAD, 1], f32, kind="ExternalInput")
    out = nc.dram_tensor("out", [NCLS_PAD, PAD], f32, kind="ExternalOutput")

    with tile.TileContext(nc) as tc:
        with (
            tc.tile_pool(name="const", bufs=1) as cp,
            tc.tile_pool(name="io", bufs=3) as io,
            tc.tile_pool(name="gp", bufs=3, space="PSUM") as gp,
            tc.tile_pool(name="lp", bufs=2, space="PSUM") as lp,
        ):
            hidx_sb = cp.tile([128, PAD // 16], mybir.dt.int16)
            nc.sync.dma_start(hidx_sb[:], hidx[:])
            tidx_sb = cp.tile([128, PAD // 16], mybir.dt.int16)
            nc.sync.dma_start(tidx_sb[:], tidx[:])
            w12_sb = cp.tile([128, KC, REP], b16)
            nc.sync.dma_start(w12_sb[:], w12[:].rearrange("(c p) n -> p c n", p=128))
            wec_sb = cp.tile([128, KC, NCLS_PAD], b16)
            nc.sync.dma_start(wec_sb[:], wec[:].rearrange("(c p) n -> p c n", p=128))
            wc_sb = cp.tile([128, KC, NCLS_PAD], b16)
            nc.sync.dma_start(wc_sb[:], wc12[:].rearrange("(c p) n -> p c n", p=128))
            bg_sb = cp.tile([128, GF], f32)
            nc.sync.dma_start(bg_sb[:], bg[:])
            bl_sb = cp.tile([NCLS_PAD, 1], f32)
            nc.sync.dma_start(bl_sb[:], bl[:])

            for t in range(T):
                gh = io.tile([128, HID // 128, R], b16, tag="gh")
                nc.gpsimd.dma_gather(
                    out_ap=gh[:], in_ap=edge[:],
                    idxs_ap=hidx_sb[:, ts(t, R // 16)],
                    num_idxs=R, num_idxs_reg=R, elem_size=HID, transpose=True,
                )
                gt = io.tile([128, HID // 128, R], b16, tag="gt")
                nc.gpsimd.dma_gather(
                    out_ap=gt[:], in_ap=edge[:],
                    idxs_ap=tidx_sb[:, ts(t, R // 16)],
                    num_idxs=R, num_idxs_reg=R, elem_size=HID, transpose=True,
                )
                u_sb = io.tile([128, GF, R], b16, tag="u")
                nc.sync.dma_start(u_sb[:], union_t[t])

                def prod_chunk(kc):
                    return gh[:, kc, :] if kc < HID // 128 else gt[:, kc - HID // 128, :]

                gated = io.tile([128, GF, R], b16, tag="gated")
                for gf in range(GF):
                    gps = gp.tile([128, R], f32, tag="gps", space="PSUM")
                    for kc in range(KC):
                        nc.tensor.matmul(
                            gps[:], w12_sb[:, kc, ds(gf * 128, 128)], prod_chunk(kc),
                            start=(kc == 0), stop=(kc == KC - 1),
                        )
                    nc.scalar.activation(
                        gated[:, gf, :], gps[:],
                        mybir.ActivationFunctionType.Identity,
                        bias=bg_sb[:, gf:gf + 1],
                    )
                    nc.vector.tensor_mul(gated[:, gf, :], gated[:, gf, :], u_sb[:, gf, :])

                lps = lp.tile([NCLS_PAD, R], f32, tag="lps", space="PSUM")
                for kc in range(KC):
                    nc.tensor.matmul(
                        lps[:], wec_sb[:, kc, :], gated[:, kc, :],
                        start=(kc == 0), stop=False,
                    )
                for kc in range(KC):
                    nc.tensor.matmul(
                        lps[:], wc_sb[:, kc, :], prod_chunk(kc),
                        start=False, stop=(kc == KC - 1),
                    )
                out_sb = io.tile([NCLS_PAD, R], f32, tag="osb")
                nc.scalar.activation(
                    out_sb[:], lps[:],
                    mybir.ActivationFunctionType.Identity, bias=bl_sb[:],
                )
                nc.sync.dma_start(out[:, ts(t, R)], out_sb[:])
    nc.compile()
    return nc


_NC_CACHE = None


def _get_nc():
    global _NC_CACHE
    if _NC_CACHE is None:
        _NC_CACHE = _build()
    return _NC_CACHE


def _wrap_idx(idx):
    """[PAD] int -> [128, PAD//16] int16 in dma_gather wrapped layout."""
    x = idx.reshape(T, R // 16, 16).transpose(2, 0, 1).reshape(16, PAD // 16)
    return np.ascontiguousarray(np.tile(x, (8, 1))).astype(np.int16)


def prepare_in_maps(edge_ctx, union_feat, W_pe, b_pe, W_pc, b_pc, W_ec, b_ec,
                    W_cl, b_cl, pair_idx):
    edge_ctx = np.asarray(edge_ctx, np.float32)
    union_feat = np.asarray(union_feat, np.float32)
    pair_idx = np.asarray(pair_idx)
    W_pe = np.asarray(W_pe, np.float32); b_pe = np.asarray(b_pe, np.float32)
    W_pc = np.asarray(W_pc, np.float32); b_pc = np.asarray(b_pc, np.float32)
    W_ec = np.asarray(W_ec, np.float32); b_ec = np.asarray(b_ec, np.float32)
    W_cl = np.asarray(W_cl, np.float32); b_cl = np.asarray(b_cl, np.float32)

    # fold W_pe / b_pe into the downstream weights
    Wpe_h, Wpe_t = W_pe[:, :HID], W_pe[:, HID:]
    W12 = np.concatenate([Wpe_h @ W_pc[:HID], Wpe_t @ W_pc[HID:]], 0)   # [1024,1024]
    Wc12 = np.concatenate([Wpe_h @ W_cl[:HID], Wpe_t @ W_cl[HID:]], 0)  # [1024,51]
    bg = b_pe[:HID] @ W_pc[:HID] + b_pe[HID:] @ W_pc[HID:] + b_pc       # [1024]
    bl = b_pe[:HID] @ W_cl[:HID] + b_pe[HID:] @ W_cl[HID:] + b_ec + b_cl  # [51]

    wec_p = np.zeros((REP, NCLS_PAD), np.float32); wec_p[:, :NCLS] = W_ec
    wc_p = np.zeros((REP, NCLS_PAD), np.float32); wc_p[:, :NCLS] = Wc12
    bl_p = np.zeros((NCLS_PAD, 1), np.float32); bl_p[:NCLS, 0] = bl

    common = {
        "edge": edge_ctx.astype(bf16),
        "w12": W12.astype(bf16),
        "wec": wec_p.astype(bf16),
        "wc12": wc_p.astype(bf16),
        "bg": np.ascontiguousarray(bg.reshape(GF, 128).T),
        "bl": bl_p,
    }

    in_maps = []
    for c in range(NCORES):
        sl = slice(c * SHARD, (c + 1) * SHARD)
        pi = pair_idx[sl]
        hi = np.zeros(PAD, np.int64); hi[:SHARD] = pi[:, 0]
        ti = np.zeros(PAD, np.int64); ti[:SHARD] = pi[:, 1]
        u = union_feat[sl].astype(bf16)
        u_pad = np.zeros((PAD, REP), bf16)
        u_pad[:SHARD] = u
        # [T, 128, GF, R] with u_t[t, p, f, r] = u[t*R + r, f*128 + p]
        u_t = np.ascontiguousarray(
            u_pad.reshape(T, R, GF, 128).transpose(0, 3, 2, 1))
        in_maps.append({
            **common,
            "hidx": _wrap_idx(hi),
            "tidx": _wrap_idx(ti),
            "union_t": u_t,
        })
    return in_maps


def kernel(**inputs):
    in_maps = prepare_in_maps(**inputs)
    nc = _get_nc()
    res = run_bass_kernel_spmd(nc, in_maps, core_ids=list(range(NCORES)))
    global LAST_RESULTS
    LAST_RESULTS = res

    out = np.empty((N_REL, NCLS), np.float32)
    for c in range(NCORES):
        out[c * SHARD:(c + 1) * SHARD] = res.results[c]["out"][:NCLS, :SHARD].T
    return out


# revision 7
# speedup vs baseline: 1.5591x; 1.5591x over previous
"""Trainium2 Bass kernel for the scene-graph relation predictor.

Math (reference):
    er   = (edge_ctx @ W_pe + b_pe)            # [N_obj, 1024]
    head = er[:, :512]; tail = er[:, 512:]
    prod = [head[i], tail[j]]                  # [N_rel, 1024]
    gate = prod @ W_pc + b_pc
    out  = (gate * union) @ W_ec + b_ec + prod @ W_cl + b_cl

Device algebra: fold W_pe into W_pc / W_cl on the host so the kernel only
needs raw edge_ctx rows:
    gate = edge[i] @ W1 + edge[j] @ W2 + bg    with W12 = [[W1],[W2]]
    out  = (gate * union) @ W_ec + edge[i] @ Wc1 + edge[j] @ Wc2 + bl

Sharding: data-parallel over the relation dim across 8 cores. edge_ctx and
the (combined) weights are replicated; union_feat / pair_idx / output rows
are split. All per-relation tensors live K-major ([feature, rel]) on chip:
gathers use dma_gather(transpose=True), union is pre-transposed host-side,
so no on-chip transposes are needed anywhere. Everything streams through
the tensor engine in bf16 with fp32 PSUM accumulation.
"""
import numpy as np
import ml_dtypes

import concourse.bass as bass
import concourse.mybir as mybir
import concourse.tile as tile
from concourse import bacc
from concourse.bass import ts, ds
from concourse.bass_utils import run_bass_kernel_spmd

bf16 = ml_dtypes.bfloat16

N_OBJ = 20000
HID = 512
REP = 1024
NCLS = 51
NCLS_PAD = 64
N_REL = 150000
NCORES = 8
SHARD = N_REL // NCORES          # 18750
R = 512                          # relations per tile
T = (SHARD + R - 1) // R         # 37 tiles
PAD = T * R                      # 18944 padded relations per core
KC = REP // 128                  # 8 contraction chunks of the 1024 dim
GF = REP // 128                  # 8 gate-feature chunks


def _build(rep=1):
    nc = bacc.Bacc(None, target_bir_lowering=False)
    f32 = mybir.dt.float32
    b16 = mybir.dt.bfloat16

    edge = nc.dram_tensor("edge", [N_OBJ, HID], b16, kind="ExternalInput")
    hidx = nc.dram_tensor("hidx", [128, PAD // 16], mybir.dt.int16, kind="ExternalInput")
    tidx = nc.dram_tensor("tidx", [128, PAD // 16], mybir.dt.int16, kind="ExternalInput")
    union_t = nc.dram_tensor("union_t", [T, 128, GF, R], b16, kind="ExternalInput")
    w12 = nc.dram_tensor("w12", [REP, REP], b16, kind="ExternalInput")
    wec = nc.dram_tensor("wec", [REP, NCLS_PAD], b16, kind="ExternalInput")
    wc12 = nc.dram_tensor("wc12", [REP, NCLS_PAD], b16, kind="ExternalInput")
    bg = nc.dram_tensor("bg", [128, GF], f32, kind="ExternalInput")
    bl = nc.dram_tensor("bl", [NCLS_PAD, 1], f32, kind="ExternalInput")
    out = nc.dram_tensor("out", [NCLS_PAD, PAD], f32, kind="ExternalOutput")

    with tile.TileContext(nc) as tc:
        with (
            tc.tile_pool(name="const", bufs=1) as cp,
            tc.tile_pool(name="io", bufs=3) as io,
            tc.tile_pool(name="gp", bufs=3, space="PSUM") as gp,
            tc.tile_pool(name="lp", bufs=2, space="PSUM") as lp,
        ):
            hidx_sb = cp.tile([128, PAD // 16], mybir.dt.int16)
            nc.sync.dma_start(hidx_sb[:], hidx[:])
            tidx_sb = cp.tile([128, PAD // 16], mybir.dt.int16)
            nc.sync.dma_start(tidx_sb[:], tidx[:])
            w12_sb = cp.tile([128, KC, REP], b16)
            nc.sync.dma_start(w12_sb[:], w12[:].rearrange("(c p) n -> p c n", p=128))
            wec_sb = cp.tile([128, KC, NCLS_PAD], b16)
            nc.sync.dma_start(wec_sb[:], wec[:].rearrange("(c p) n -> p c n", p=128))
            wc_sb = cp.tile([128, KC, NCLS_PAD], b16)
            nc.sync.dma_start(wc_sb[:], wc12[:].rearrange("(c p) n -> p c n", p=128))
            bg_sb = cp.tile([128, GF], f32)
            nc.sync.dma_start(bg_sb[:], bg[:])
            bl_sb = cp.tile([NCLS_PAD, 1], f32)
            nc.sync.dma_start(bl_sb[:], bl[:])

            def tile_body(t):
                gh = io.tile([128, HID // 128, R], b16, tag="gh", name="gh")
                nc.gpsimd.dma_gather(
                    out_ap=gh[:], in_ap=edge[:],
                    idxs_ap=hidx_sb[:, ts(t, R // 16)],
                    num_idxs=R, num_idxs_reg=R, elem_size=HID, transpose=True,
                )
                gt = io.tile([128, HID // 128, R], b16, tag="gt", name="gt")
                nc.gpsimd.dma_gather(
                    out_ap=gt[:], in_ap=edge[:],
                    idxs_ap=tidx_sb[:, ts(t, R // 16)],
                    num_idxs=R, num_idxs_reg=R, elem_size=HID, transpose=True,
                )
                u_sb = io.tile([128, GF, R], b16, tag="u", name="u_sb")
                nc.sync.dma_start(u_sb[:], union_t[t])

                def prod_chunk(kc):
                    if kc < HID // 128:
                        return gh[:, kc, :]
                    return gt[:, kc - HID // 128, :]

                gated = io.tile([128, GF, R], b16, tag="gated", name="gated")
                for gf in range(GF):
                    gps = gp.tile([128, R], f32, tag="gps", space="PSUM", name="gps")
                    for kc in range(KC):
                        nc.tensor.matmul(
                            gps[:], w12_sb[:, kc, ds(gf * 128, 128)], prod_chunk(kc),
                            start=(kc == 0), stop=(kc == KC - 1),
                        )
                    nc.scalar.activation(
                        gated[:, gf, :], gps[:],
                        mybir.ActivationFunctionType.Identity,
                        bias=bg_sb[:, gf:gf + 1],
                    )
                    nc.vector.tensor_mul(gated[:, gf, :], gated[:, gf, :], u_sb[:, gf, :])

                lps = lp.tile([NCLS_PAD, R], f32, tag="lps", space="PSUM", name="lps")
                for kc in range(KC):
                    nc.tensor.matmul(
                        lps[:], wec_sb[:, kc, :], gated[:, kc, :],
                        start=(kc == 0), stop=False,
                    )
                for kc in range(KC):
                    nc.tensor.matmul(
                        lps[:], wc_sb[:, kc, :], prod_chunk(kc),
                        start=False, stop=(kc == KC - 1),
                    )
                out_sb = io.tile([NCLS_PAD, R], f32, tag="osb", name="out_sb")
                nc.scalar.activation(
                    out_sb[:], lps[:],
                    mybir.ActivationFunctionType.Identity, bias=bl_sb[:],
                )
                nc.sync.dma_start(out[:, ts(t, R)], out_sb[:])

            if rep == 1:
                for t in range(T):
                    tile_body(t)
            else:
                with tc.For_i(0, rep, 1):
                    for t in range(T):
                        tile_body(t)
    nc.compile()
    return nc


_NC_CACHE = None


def _get_nc():
    global _NC_CACHE
    if _NC_CACHE is None:
        _NC_CACHE = _build()
    return _NC_CACHE


def _wrap_idx(idx):
    """[PAD] int -> [128, PAD//16] int16 in dma_gather wrapped layout."""
    x = idx.reshape(T, R // 16, 16).transpose(2, 0, 1).reshape(16, PAD // 16)
    return np.ascontiguousarray(np.tile(x, (8, 1))).astype(np.int16)


def prepare_in_maps(edge_ctx, union_feat, W_pe, b_pe, W_pc, b_pc, W_ec, b_ec,
                    W_cl, b_cl, pair_idx):
    edge_ctx = np.asarray(edge_ctx, np.float32)
    union_feat = np.asarray(union_feat, np.float32)
    pair_idx = np.asarray(pair_idx)
    W_pe = np.asarray(W_pe, np.float32); b_pe = np.asarray(b_pe, np.float32)
    W_pc = np.asarray(W_pc, np.float32); b_pc = np.asarray(b_pc, np.float32)
    W_ec = np.asarray(W_ec, np.float32); b_ec = np.asarray(b_ec, np.float32)
    W_cl = np.asarray(W_cl, np.float32); b_cl = np.asarray(b_cl, np.float32)

    # fold W_pe / b_pe into the downstream weights
    Wpe_h, Wpe_t = W_pe[:, :HID], W_pe[:, HID:]
    W12 = np.concatenate([Wpe_h @ W_pc[:HID], Wpe_t @ W_pc[HID:]], 0)   # [1024,1024]
    Wc12 = np.concatenate([Wpe_h @ W_cl[:HID], Wpe_t @ W_cl[HID:]], 0)  # [1024,51]
    bg = b_pe[:HID] @ W_pc[:HID] + b_pe[HID:] @ W_pc[HID:] + b_pc       # [1024]
    bl = b_pe[:HID] @ W_cl[:HID] + b_pe[HID:] @ W_cl[HID:] + b_ec + b_cl  # [51]

    wec_p = np.zeros((REP, NCLS_PAD), np.float32); wec_p[:, :NCLS] = W_ec
    wc_p = np.zeros((REP, NCLS_PAD), np.float32); wc_p[:, :NCLS] = Wc12
    bl_p = np.zeros((NCLS_PAD, 1), np.float32); bl_p[:NCLS, 0] = bl

    common = {
        "edge": edge_ctx.astype(bf16),
        "w12": W12.astype(bf16),
        "wec": wec_p.astype(bf16),
        "wc12": wc_p.astype(bf16),
        "bg": np.ascontiguousarray(bg.reshape(GF, 128).T),
        "bl": bl_p,
    }

    in_maps = []
    for c in range(NCORES):
        sl = slice(c * SHARD, (c + 1) * SHARD)
        pi = pair_idx[sl]
        hi = np.zeros(PAD, np.int64); hi[:SHARD] = pi[:, 0]
        ti = np.zeros(PAD, np.int64); ti[:SHARD] = pi[:, 1]
        u = union_feat[sl].astype(bf16)
        u_pad = np.zeros((PAD, REP), bf16)
        u_pad[:SHARD] = u
        # [T, 128, GF, R] with u_t[t, p, f, r] = u[t*R + r, f*128 + p]
        u_t = np.ascontiguousarray(
            u_pad.reshape(T, R, GF, 128).transpose(0, 3, 2, 1))
        in_maps.append({
            **common,
            "hidx": _wrap_idx(hi),
            "tidx": _wrap_idx(ti),
            "union_t": u_t,
        })
    return in_maps


def kernel(**inputs):
    in_maps = prepare_in_maps(**inputs)
    nc = _get_nc()
    res = run_bass_kernel_spmd(nc, in_maps, core_ids=list(range(NCORES)))
    global LAST_RESULTS
    LAST_RESULTS = res

    out = np.empty((N_REL, NCLS), np.float32)
    for c in range(NCORES):
        out[c * SHARD:(c + 1) * SHARD] = res.results[c]["out"][:NCLS, :SHARD].T
    return out


# revision 8
# speedup vs baseline: 2.4100x; 1.5458x over previous
"""Trainium2 Bass kernel for the scene-graph relation predictor.

Math (reference):
    er   = (edge_ctx @ W_pe + b_pe)            # [N_obj, 1024]
    head = er[:, :512]; tail = er[:, 512:]
    prod = [head[i], tail[j]]                  # [N_rel, 1024]
    gate = prod @ W_pc + b_pc
    out  = (gate * union) @ W_ec + b_ec + prod @ W_cl + b_cl

Device algebra: fold W_pe into W_pc / W_cl on the host:
    gate = edge[i] @ W1 + edge[j] @ W2 + bg
    out  = (gate * union) @ W_ec + edge[i] @ Wc1 + edge[j] @ Wc2 + bl

Sharding: relations are sorted by head index on the host and split into 8
equal shards, so each core's heads fall in a ~2500-object contiguous range.
The core precomputes a table H'[o] = edge[o] @ [W1 | Wc1] for its range
(~60 us), then per relation just gathers H' rows — only the tail side needs
per-relation matmuls. All per-relation tensors are K-major ([feature, rel]):
gathers use dma_gather(transpose=True) and union is pre-transposed host-side,
so there are no on-chip transposes. bf16 throughout with fp32 PSUM.
"""
import numpy as np
import ml_dtypes

import concourse.bass as bass
import concourse.mybir as mybir
import concourse.tile as tile
from concourse import bacc
from concourse.bass import ts, ds
from concourse.bass_utils import run_bass_kernel_spmd

bf16 = ml_dtypes.bfloat16

N_OBJ = 20000
HID = 512
REP = 1024
NCLS = 51
NCLS_PAD = 64
N_REL = 150000
NCORES = 8
SHARD = N_REL // NCORES          # 18750
R = 512                          # relations per tile
T = (SHARD + R - 1) // R         # 37 tiles
PAD = T * R                      # 18944 padded relations per core
KH = HID // 128                  # 4 contraction chunks of the 512 dim
GF = REP // 128                  # 8 gate-feature chunks
OBJ_PAD = 3072                   # per-core head-range table rows
OT = OBJ_PAD // 128              # 24 table build tiles
HP_W = REP + 2 * NCLS_PAD        # 1152 = gate 1024 + cls 64 + pad 64
HC = HP_W // 128                 # 9 gathered chunks per H' row


def _build(rep=1):
    nc = bacc.Bacc(None, target_bir_lowering=False)
    f32 = mybir.dt.float32
    b16 = mybir.dt.bfloat16

    edge = nc.dram_tensor("edge", [N_OBJ, HID], b16, kind="ExternalInput")
    edget = nc.dram_tensor("edget", [HID, OBJ_PAD], b16, kind="ExternalInput")
    hidx = nc.dram_tensor("hidx", [128, PAD // 16], mybir.dt.int16, kind="ExternalInput")
    tidx = nc.dram_tensor("tidx", [128, PAD // 16], mybir.dt.int16, kind="ExternalInput")
    union_t = nc.dram_tensor("union_t", [T, 128, GF, R], b16, kind="ExternalInput")
    w1p = nc.dram_tensor("w1p", [HID, HP_W], b16, kind="ExternalInput")
    w2 = nc.dram_tensor("w2", [HID, REP], b16, kind="ExternalInput")
    wc2 = nc.dram_tensor("wc2", [HID, NCLS_PAD], b16, kind="ExternalInput")
    wec = nc.dram_tensor("wec", [REP, NCLS_PAD], b16, kind="ExternalInput")
    bg = nc.dram_tensor("bg", [128, GF], f32, kind="ExternalInput")
    bl = nc.dram_tensor("bl", [NCLS_PAD, 1], f32, kind="ExternalInput")
    out = nc.dram_tensor("out", [NCLS_PAD, PAD], f32, kind="ExternalOutput")

    with tile.TileContext(nc) as tc:
        with (
            tc.tile_pool(name="const", bufs=1) as cp,
            tc.tile_pool(name="dram", bufs=1, space="DRAM") as dp,
            tc.tile_pool(name="bld", bufs=2) as bp,
            tc.tile_pool(name="io", bufs=3) as io,
            tc.tile_pool(name="bps", bufs=2, space="PSUM") as bps,
            tc.tile_pool(name="gp", bufs=3, space="PSUM") as gp,
            tc.tile_pool(name="lp", bufs=2, space="PSUM") as lp,
        ):
            hidx_sb = cp.tile([128, PAD // 16], mybir.dt.int16)
            nc.sync.dma_start(hidx_sb[:], hidx[:])
            tidx_sb = cp.tile([128, PAD // 16], mybir.dt.int16)
            nc.sync.dma_start(tidx_sb[:], tidx[:])
            et_sb = cp.tile([128, KH, OBJ_PAD], b16)
            nc.sync.dma_start(et_sb[:], edget[:].rearrange("(c p) n -> p c n", p=128))
            w1p_sb = cp.tile([128, KH, HP_W], b16)
            nc.sync.dma_start(w1p_sb[:], w1p[:].rearrange("(c p) n -> p c n", p=128))
            w2_sb = cp.tile([128, KH, REP], b16)
            nc.sync.dma_start(w2_sb[:], w2[:].rearrange("(c p) n -> p c n", p=128))
            wc2_sb = cp.tile([128, KH, NCLS_PAD], b16)
            nc.sync.dma_start(wc2_sb[:], wc2[:].rearrange("(c p) n -> p c n", p=128))
            wec_sb = cp.tile([128, GF, NCLS_PAD], b16)
            nc.sync.dma_start(wec_sb[:], wec[:].rearrange("(c p) n -> p c n", p=128))
            bg_sb = cp.tile([128, GF], f32)
            nc.sync.dma_start(bg_sb[:], bg[:])
            bl_sb = cp.tile([NCLS_PAD, 1], f32)
            nc.sync.dma_start(bl_sb[:], bl[:])

            htable = dp.tile([OBJ_PAD, HP_W], b16, space="DRAM")

            def build_table():
                # H'[o, :] = edge[o] @ [W1 | Wc1 | 0] for the core's head range
                for ot in range(OT):
                    htile = bp.tile([128, HP_W], b16, tag="ht", name="htile")
                    for pc0, pcw in ((0, 512), (512, 512), (1024, 128)):
                        hp = bps.tile([128, pcw], f32, tag="hp", space="PSUM",
                                      name="hp", padded_shape=[128, 512])
                        for kc in range(KH):
                            nc.tensor.matmul(
                                hp[:], et_sb[:, kc, ts(ot, 128)],
                                w1p_sb[:, kc, ds(pc0, pcw)],
                                start=(kc == 0), stop=(kc == KH - 1),
                            )
                        nc.scalar.activation(
                            htile[:, ds(pc0, pcw)], hp[:],
                            mybir.ActivationFunctionType.Copy,
                        )
                    nc.sync.dma_start(htable[ts(ot, 128), :], htile[:])

            def tile_body(t):
                gh = io.tile([128, HC, R], b16, tag="gh", name="gh")
                nc.gpsimd.dma_gather(
                    out_ap=gh[:], in_ap=htable[:],
                    idxs_ap=hidx_sb[:, ts(t, R // 16)],
                    num_idxs=R, num_idxs_reg=R, elem_size=HP_W, transpose=True,
                )
                gt = io.tile([128, KH, R], b16, tag="gt", name="gt")
                nc.gpsimd.dma_gather(
                    out_ap=gt[:], in_ap=edge[:],
                    idxs_ap=tidx_sb[:, ts(t, R // 16)],
                    num_idxs=R, num_idxs_reg=R, elem_size=HID, transpose=True,
                )
                u_sb = io.tile([128, GF, R], b16, tag="u", name="u_sb")
                nc.sync.dma_start(u_sb[:], union_t[t])

                gated = io.tile([128, GF, R], b16, tag="gated", name="gated")
                for gf in range(GF):
                    gps = gp.tile([128, R], f32, tag="gps", space="PSUM", name="gps")
                    for kc in range(KH):
                        nc.tensor.matmul(
                            gps[:], w2_sb[:, kc, ds(gf * 128, 128)], gt[:, kc, :],
                            start=(kc == 0), stop=(kc == KH - 1),
                        )
                    # gate = (tail_psum + bg) + H'_gathered, then * union
                    nc.scalar.activation(
                        gated[:, gf, :], gps[:],
                        mybir.ActivationFunctionType.Identity,
                        bias=bg_sb[:, gf:gf + 1],
                    )
                    nc.vector.tensor_add(gated[:, gf, :], gated[:, gf, :], gh[:, gf, :])
                    nc.vector.tensor_mul(gated[:, gf, :], gated[:, gf, :], u_sb[:, gf, :])

                lps = lp.tile([NCLS_PAD, R], f32, tag="lps", space="PSUM", name="lps")
                for kc in range(GF):
                    nc.tensor.matmul(
                        lps[:], wec_sb[:, kc, :], gated[:, kc, :],
                        start=(kc == 0), stop=False,
                    )
                for kc in range(KH):
                    nc.tensor.matmul(
                        lps[:], wc2_sb[:, kc, :], gt[:, kc, :],
                        start=False, stop=(kc == KH - 1),
                    )
                out_sb = io.tile([NCLS_PAD, R], f32, tag="osb", name="out_sb")
                nc.scalar.activation(
                    out_sb[:], lps[:],
                    mybir.ActivationFunctionType.Identity, bias=bl_sb[:],
                )
                # + head classifier contribution from the gathered H' rows
                nc.vector.tensor_add(out_sb[:], out_sb[:], gh[:NCLS_PAD, GF, :])
                nc.sync.dma_start(out[:, ts(t, R)], out_sb[:])

            def whole():
                build_table()
                for t in range(T):
                    tile_body(t)

            if rep == 1:
                whole()
            else:
                with tc.For_i(0, rep, 1):
                    whole()
    nc.compile()
    return nc


_NC_CACHE = None


def _get_nc():
    global _NC_CACHE
    if _NC_CACHE is None:
        _NC_CACHE = _build()
    return _NC_CACHE


def _wrap_idx(idx):
    """[PAD] int -> [128, PAD//16] int16 in dma_gather wrapped layout."""
    x = idx.reshape(T, R // 16, 16).transpose(2, 0, 1).reshape(16, PAD // 16)
    return np.ascontiguousarray(np.tile(x, (8, 1))).astype(np.int16)


def prepare_in_maps(edge_ctx, union_feat, W_pe, b_pe, W_pc, b_pc, W_ec, b_ec,
                    W_cl, b_cl, pair_idx):
    edge_ctx = np.asarray(edge_ctx, np.float32)
    union_feat = np.asarray(union_feat, np.float32)
    pair_idx = np.asarray(pair_idx)
    W_pe = np.asarray(W_pe, np.float32); b_pe = np.asarray(b_pe, np.float32)
    W_pc = np.asarray(W_pc, np.float32); b_pc = np.asarray(b_pc, np.float32)
    W_ec = np.asarray(W_ec, np.float32); b_ec = np.asarray(b_ec, np.float32)
    W_cl = np.asarray(W_cl, np.float32); b_cl = np.asarray(b_cl, np.float32)

    # fold W_pe / b_pe into the downstream weights
    Wpe_h, Wpe_t = W_pe[:, :HID], W_pe[:, HID:]
    W1 = Wpe_h @ W_pc[:HID]          # [512, 1024] head gate
    W2 = Wpe_t @ W_pc[HID:]          # [512, 1024] tail gate
    Wc1 = Wpe_h @ W_cl[:HID]         # [512, 51]   head cls
    Wc2 = Wpe_t @ W_cl[HID:]         # [512, 51]   tail cls
    bg = b_pe[:HID] @ W_pc[:HID] + b_pe[HID:] @ W_pc[HID:] + b_pc         # [1024]
    bl = b_pe[:HID] @ W_cl[:HID] + b_pe[HID:] @ W_cl[HID:] + b_ec + b_cl  # [51]

    w1p = np.zeros((HID, HP_W), np.float32)
    w1p[:, :REP] = W1
    w1p[:, REP:REP + NCLS] = Wc1
    wc2_p = np.zeros((HID, NCLS_PAD), np.float32); wc2_p[:, :NCLS] = Wc2
    wec_p = np.zeros((REP, NCLS_PAD), np.float32); wec_p[:, :NCLS] = W_ec
    bl_p = np.zeros((NCLS_PAD, 1), np.float32); bl_p[:NCLS, 0] = bl

    ec_b = edge_ctx.astype(bf16)
    common = {
        "edge": ec_b,
        "w1p": w1p.astype(bf16),
        "w2": W2.astype(bf16),
        "wc2": wc2_p.astype(bf16),
        "wec": wec_p.astype(bf16),
        "bg": np.ascontiguousarray(bg.reshape(GF, 128).T),
        "bl": bl_p,
    }

    # sort relations by head so each core's heads are a contiguous range
    perm = np.argsort(pair_idx[:, 0], kind="stable")
    pi_s = pair_idx[perm]

    in_maps = []
    for c in range(NCORES):
        sl = slice(c * SHARD, (c + 1) * SHARD)
        pi = pi_s[sl]
        lo = int(pi[0, 0])
        span = int(pi[-1, 0]) - lo + 1
        assert span <= OBJ_PAD, f"core {c} head range {span} > {OBJ_PAD}"
        et = np.zeros((HID, OBJ_PAD), bf16)
        n = min(OBJ_PAD, N_OBJ - lo)
        et[:, :n] = ec_b[lo:lo + n].T

        hi = np.zeros(PAD, np.int64); hi[:SHARD] = pi[:, 0] - lo
        ti = np.zeros(PAD, np.int64); ti[:SHARD] = pi[:, 1]
        u = union_feat[perm[sl]].astype(bf16)
        u_pad = np.zeros((PAD, REP), bf16)
        u_pad[:SHARD] = u
        # [T, 128, GF, R] with u_t[t, p, f, r] = u[t*R + r, f*128 + p]
        u_t = np.ascontiguousarray(
            u_pad.reshape(T, R, GF, 128).transpose(0, 3, 2, 1))
        in_maps.append({
            **common,
            "edget": et,
            "hidx": _wrap_idx(hi),
            "tidx": _wrap_idx(ti),
            "union_t": u_t,
        })
    return in_maps, perm


def kernel(**inputs):
    in_maps, perm = prepare_in_maps(**inputs)
    nc = _get_nc()
    res = run_bass_kernel_spmd(nc, in_maps, core_ids=list(range(NCORES)))
    global LAST_RESULTS
    LAST_RESULTS = res

    out = np.empty((N_REL, NCLS), np.float32)
    for c in range(NCORES):
        out[perm[c * SHARD:(c + 1) * SHARD]] = res.results[c]["out"][:NCLS, :SHARD].T
    return out
